# revision 4
# baseline (speedup 1.0000x reference)
"""AbsPosSelfAttention Trainium2 kernel (8 NeuronCores, SPMD).

Problem: q,k,v [2, 8, 64, 64, 32] f32, emb_h/emb_w [64, 32] f32.
  logits[b,n,x,y,p,q] = q.(k + emb)/sqrt(d);  weights = softmax over (p,q);
  out[b,x,y,n*32+32] = sum weights * v  -> [2, 64, 64, 256] f32.

Key identity: q.k + q.emb = q.(k+emb), so this is plain attention with
k' = k + emb (emb[p,q,:] = emb_h[p,:] + emb_w[q,:], shared by all (b,n)).

Sharding: 16 (batch, head) pairs across 8 cores -> 2 pairs/core. Per pair the
kernel computes, with S = 4096, D = 32:
  S^T tiles:  ST[kv, x] = k'ized K @ Q^T via PE (bf16, K-dim = 32)
  P^T = exp(ST / sqrt(32)) on ACT (bf16 out, scale folded into ACT's affine)
  O^T accum:  lhsT = [v | ones] [128kv, 33] -> OT[33, x] in PSUM (f32), the
              ones column produces the softmax denominators for free
  PE-transpose OT -> [128x, 33], normalize rows by 1/denominator on DVE, store.

Host side only reshapes/shards (chunk-major [128, 1024] tiles) and computes the
tiny [4096, 32] emb table; all O(S^2) math runs on-device.
"""

import numpy as np

BS, HEADS, H, W, DIM = 2, 8, 64, 64, 32
SEQ = H * W                      # 4096
N_CORES = 8
PPC = (BS * HEADS) // N_CORES    # pairs per core = 2
P = 128                          # partitions
CHUNKS = SEQ // P                # 32 kv chunks of 128
NB = 512                         # x-block width
XBLOCKS = SEQ // NB              # 8
GROUP = 2                        # kv chunks per matmul1/exp group
SCALE = float(DIM) ** -0.5

_RUNNER_CACHE = {}


def _split_excess_waits(nc, mybir, limit=1):
    """This container's walrus rejects >~2 sync waits on one instruction.
    Move excess waits onto same-engine NOPs inserted right before the
    instruction (same-queue ordering makes this semantically identical)."""
    n_split = 0
    for fn in nc.m.functions:
        for blk in fn.blocks:
            new_insts = []
            for inst in blk.instructions:
                si = inst.sync_info
                if si is not None and si.on_wait and len(si.on_wait) > limit:
                    waits = list(si.on_wait)
                    extra, keep = waits[:-limit], waits[-limit:]
                    for j, wt in enumerate(extra):
                        nop = mybir.InstNoOp(
                            name=f"{inst.name}-wsplit{j}", ins=[], outs=[]
                        )
                        nop.engine = inst.engine
                        nop.sync_info = mybir.SyncInfo(on_wait=[wt], on_update=[])
                        new_insts.append(nop)
                        n_split += 1
                    inst.sync_info = mybir.SyncInfo(
                        on_wait=keep, on_update=list(si.on_update or [])
                    )
                new_insts.append(inst)
            blk.instructions[:] = new_insts
    return n_split


def _build_nc():
    import concourse.bass as bass
    import concourse.tile as tile
    from concourse import mybir
    from concourse.masks import make_identity

    f32 = mybir.dt.float32
    bf16 = mybir.dt.bfloat16

    nc = bass.Bass()
    dq = nc.declare_dram_parameter("q", [PPC, P, CHUNKS * DIM], f32, isOutput=False)
    dk = nc.declare_dram_parameter("k", [PPC, P, CHUNKS * DIM], f32, isOutput=False)
    dv = nc.declare_dram_parameter("v", [PPC, P, CHUNKS * DIM], f32, isOutput=False)
    de = nc.declare_dram_parameter("emb", [P, CHUNKS * DIM], f32, isOutput=False)
    do = nc.declare_dram_parameter("o", [PPC, SEQ, DIM], f32, isOutput=True)

    with tile.TileContext(nc) as tc:
        import contextlib

        with contextlib.ExitStack() as ctx:
            consts = ctx.enter_context(tc.tile_pool(name="consts", bufs=1))
            loads = ctx.enter_context(tc.tile_pool(name="loads", bufs=2))
            bfp = ctx.enter_context(tc.tile_pool(name="bfp", bufs=2))
            tp = ctx.enter_context(tc.tile_pool(name="tp", bufs=2))
            expp = ctx.enter_context(tc.tile_pool(name="expp", bufs=2))
            otp = ctx.enter_context(tc.tile_pool(name="otp", bufs=2))
            outp = ctx.enter_context(tc.tile_pool(name="outp", bufs=2))
            rsp = ctx.enter_context(tc.tile_pool(name="rsp", bufs=2))
            ps_t = ctx.enter_context(tc.tile_pool(name="ps_t", bufs=2, space="PSUM"))
            ps_mm1 = ctx.enter_context(tc.tile_pool(name="ps_mm1", bufs=2, space="PSUM"))
            ps_ot = ctx.enter_context(tc.tile_pool(name="ps_ot", bufs=1, space="PSUM"))
            ps_out = ctx.enter_context(tc.tile_pool(name="ps_out", bufs=1, space="PSUM"))

            identb = consts.tile([P, P], bf16)
            make_identity(nc, identb)
            identf = consts.tile([P, P], f32)
            make_identity(nc, identf)
            embf = consts.tile([P, CHUNKS * DIM], f32)
            nc.sync.dma_start(out=embf, in_=de[:, :])

            for pr in range(PPC):
                # ---- load + k' = k + emb + bf16 convert (chunk-major) ----
                qf = loads.tile([P, CHUNKS * DIM], f32, tag="qf")
                kf = loads.tile([P, CHUNKS * DIM], f32, tag="kf")
                vf = loads.tile([P, CHUNKS * DIM], f32, tag="vf")
                nc.sync.dma_start(out=qf, in_=dq[pr, :, :])
                nc.sync.dma_start(out=kf, in_=dk[pr, :, :])
                nc.sync.dma_start(out=vf, in_=dv[pr, :, :])

                kpf = bfp.tile([P, CHUNKS * DIM], f32, tag="kpf")
                nc.vector.tensor_add(kpf, kf, embf)
                qb = bfp.tile([P, CHUNKS * DIM], bf16, tag="qb")
                nc.vector.tensor_copy(qb, qf)
                kb = bfp.tile([P, CHUNKS * DIM], bf16, tag="kb")
                nc.vector.tensor_copy(kb, kpf)
                # v' = [v | 1] per chunk: [128, 33*32] bf16
                vb = bfp.tile([P, CHUNKS, DIM + 1], bf16, tag="vb")
                nc.vector.memset(vb, 1.0)
                nc.vector.tensor_copy(
                    vb[:, :, 0:DIM],
                    vf.rearrange("p (c d) -> p c d", d=DIM),
                )

                # ---- PE transposes: qT/kT [32, 4096] bf16 ----
                qT = tp.tile([DIM, SEQ], bf16, tag="qT")
                kT = tp.tile([DIM, SEQ], bf16, tag="kT")
                for dst, src in ((qT, qb), (kT, kb)):
                    for half in range(4):
                        pst = ps_t.tile([DIM, P * 8], bf16, tag="pst")
                        for cc in range(8):
                            c = 8 * half + cc
                            nc.tensor.transpose(
                                pst[:, P * cc : P * (cc + 1)],
                                src[:, DIM * c : DIM * (c + 1)],
                                identb,
                            )
                        nc.vector.tensor_copy(
                            dst[:, P * 8 * half : P * 8 * (half + 1)], pst
                        )

                # ---- main loop over x blocks ----
                for w in range(XBLOCKS):
                    exb = expp.tile([P, CHUNKS * NB], bf16, tag="exb")
                    for g in range(CHUNKS // GROUP):
                        ps = ps_mm1.tile([P, GROUP * NB], mybir.dt.float32, tag="mm1")
                        for i in range(GROUP):
                            c = GROUP * g + i
                            nc.tensor.matmul(
                                ps[:, NB * i : NB * (i + 1)],
                                lhsT=kT[:, P * c : P * (c + 1)],
                                rhs=qT[:, NB * w : NB * (w + 1)],
                                start=True,
                                stop=True,
                            )
                        nc.scalar.activation(
                            exb[:, GROUP * NB * g : GROUP * NB * (g + 1)],
                            ps,
                            mybir.ActivationFunctionType.Exp,
                            scale=SCALE,
                        )

                    ot = ps_ot.tile([DIM + 1, NB], mybir.dt.float32, tag="ot")
                    exb3 = exb.rearrange("p (c n) -> p c n", n=NB)
                    for c in range(CHUNKS):
                        nc.tensor.matmul(
                            ot,
                            lhsT=vb[:, c, :],
                            rhs=exb3[:, c, :],
                            start=(c == 0),
                            stop=(c == CHUNKS - 1),
                        )
                    otsb = otp.tile([DIM + 1, NB], mybir.dt.float32, tag="otsb")
                    nc.vector.tensor_copy(otsb, ot)

                    outsb = outp.tile([P, NB // P, DIM], mybir.dt.float32, tag="outsb")
                    for s in range(NB // P):
                        po = ps_out.tile([P, DIM + 1], mybir.dt.float32, tag="po")
                        nc.tensor.transpose(
                            po,
                            otsb[:, P * s : P * (s + 1)],
                            identf[0 : DIM + 1, 0 : DIM + 1],
                        )
                        rs = rsp.tile([P, 1], mybir.dt.float32, tag="rs")
                        nc.vector.reciprocal(rs, po[:, DIM : DIM + 1])
                        nc.vector.tensor_scalar_mul(outsb[:, s, :], po[:, 0:DIM], rs)

                    nc.sync.dma_start(
                        out=do[pr, NB * w : NB * (w + 1), :].rearrange(
                            "(s p2) d -> p2 s d", p2=P
                        ),
                        in_=outsb,
                    )

    _split_excess_waits(nc, mybir)
    return nc


def _get_runner():
    if "nc" not in _RUNNER_CACHE:
        _RUNNER_CACHE["nc"] = _build_nc()
    return _RUNNER_CACHE["nc"]


def _host_prep(q, k, v, emb_h, emb_w):
    """Shard + rearrange inputs into per-core input maps."""
    q = np.ascontiguousarray(q, dtype=np.float32).reshape(BS * HEADS, SEQ, DIM)
    k = np.ascontiguousarray(k, dtype=np.float32).reshape(BS * HEADS, SEQ, DIM)
    v = np.ascontiguousarray(v, dtype=np.float32).reshape(BS * HEADS, SEQ, DIM)
    emb = (
        np.asarray(emb_h, dtype=np.float32)[:, None, :]
        + np.asarray(emb_w, dtype=np.float32)[None, :, :]
    ).reshape(SEQ, DIM)

    def cm(x):  # [..., 4096, 32] -> chunk-major [..., 128, 1024]
        lead = x.shape[:-2]
        return (
            x.reshape(*lead, CHUNKS, P, DIM)
            .swapaxes(-3, -2)
            .reshape(*lead, P, CHUNKS * DIM)
        )

    qcm, kcm, vcm, ecm = cm(q), cm(k), cm(v), cm(emb)
    in_maps = []
    for c in range(N_CORES):
        sl = slice(PPC * c, PPC * (c + 1))
        in_maps.append(
            {
                "q": np.ascontiguousarray(qcm[sl]),
                "k": np.ascontiguousarray(kcm[sl]),
                "v": np.ascontiguousarray(vcm[sl]),
                "emb": np.ascontiguousarray(ecm),
            }
        )
    return in_maps


def _host_gather(results):
    o_all = np.stack([results[c]["o"] for c in range(N_CORES)], axis=0).reshape(
        BS, HEADS, H, W, DIM
    )
    return np.ascontiguousarray(
        o_all.transpose(0, 2, 3, 1, 4).reshape(BS, H, W, HEADS * DIM)
    )


def run_on_hw(in_maps, trace=False):
    from concourse.bass_utils import run_bass_kernel_spmd

    nc = _get_runner()
    res = run_bass_kernel_spmd(nc, in_maps, core_ids=list(range(N_CORES)), trace=trace)
    return res


def kernel(q, k, v, emb_h, emb_w):
    in_maps = _host_prep(q, k, v, emb_h, emb_w)
    res = run_on_hw(in_maps, trace=False)
    return _host_gather(res.results)


# revision 6
# speedup vs baseline: 1.2986x; 1.2986x over previous
"""AbsPosSelfAttention Trainium2 kernel (8 NeuronCores, SPMD).

Problem: q,k,v [2, 8, 64, 64, 32] f32, emb_h/emb_w [64, 32] f32.
  logits[b,n,x,y,p,q] = q.(k + emb)/sqrt(d);  weights = softmax over (p,q);
  out[b,x,y,n*32+32] = sum weights * v  -> [2, 64, 64, 256] f32.

Key identity: q.k + q.emb = q.(k+emb), so this is plain attention with
k' = k + emb (emb[p,q,:] = emb_h[p,:] + emb_w[q,:], shared by all (b,n)).

Sharding: 16 (batch, head) pairs across 8 cores -> 2 pairs/core. Per pair the
kernel computes, with S = 4096, D = 32:
  S^T tiles:  ST[kv, x] = k'ized K @ Q^T via PE (bf16, K-dim = 32)
  P^T = exp(ST / sqrt(32)) on ACT (bf16 out, scale folded into ACT's affine)
  O^T accum:  lhsT = [v | ones] [128kv, 33] -> OT[33, x] in PSUM (f32), the
              ones column produces the softmax denominators for free
  PE-transpose OT -> [128x, 33], normalize rows by 1/denominator on DVE, store.

Host side only reshapes/shards (chunk-major [128, 1024] tiles) and computes the
tiny [4096, 32] emb table; all O(S^2) math runs on-device.
"""

import numpy as np

BS, HEADS, H, W, DIM = 2, 8, 64, 64, 32
SEQ = H * W                      # 4096
N_CORES = 8
PPC = (BS * HEADS) // N_CORES    # pairs per core = 2
P = 128                          # partitions
CHUNKS = SEQ // P                # 32 kv chunks of 128
NB = 512                         # x-block width
XBLOCKS = SEQ // NB              # 8
GROUP = 2                        # kv chunks per matmul1/exp group
SCALE = float(DIM) ** -0.5

_RUNNER_CACHE = {}


def _split_excess_waits(nc, mybir, limit=1):
    """This container's walrus rejects >~2 sync waits on one instruction.
    Move excess waits onto same-engine NOPs inserted right before the
    instruction (same-queue ordering makes this semantically identical)."""
    n_split = 0
    for fn in nc.m.functions:
        for blk in fn.blocks:
            new_insts = []
            for inst in blk.instructions:
                si = inst.sync_info
                if si is not None and si.on_wait and len(si.on_wait) > limit:
                    waits = list(si.on_wait)
                    extra, keep = waits[:-limit], waits[-limit:]
                    for j, wt in enumerate(extra):
                        nop = mybir.InstNoOp(
                            name=f"{inst.name}-wsplit{j}", ins=[], outs=[]
                        )
                        nop.engine = inst.engine
                        nop.sync_info = mybir.SyncInfo(on_wait=[wt], on_update=[])
                        new_insts.append(nop)
                        n_split += 1
                    inst.sync_info = mybir.SyncInfo(
                        on_wait=keep, on_update=list(si.on_update or [])
                    )
                new_insts.append(inst)
            blk.instructions[:] = new_insts
    return n_split


def _build_nc():
    import concourse.bass as bass
    import concourse.tile as tile
    from concourse import mybir
    from concourse.masks import make_identity

    f32 = mybir.dt.float32
    bf16 = mybir.dt.bfloat16

    nc = bass.Bass()
    dq = nc.declare_dram_parameter("q", [PPC, P, CHUNKS * DIM], f32, isOutput=False)
    dk = nc.declare_dram_parameter("k", [PPC, P, CHUNKS * DIM], f32, isOutput=False)
    dv = nc.declare_dram_parameter("v", [PPC, P, CHUNKS * DIM], f32, isOutput=False)
    de = nc.declare_dram_parameter("emb", [P, CHUNKS * DIM], f32, isOutput=False)
    do = nc.declare_dram_parameter("o", [PPC, SEQ, DIM], f32, isOutput=True)

    with tile.TileContext(nc) as tc:
        import contextlib

        with contextlib.ExitStack() as ctx:
            consts = ctx.enter_context(tc.tile_pool(name="consts", bufs=1))
            loads = ctx.enter_context(tc.tile_pool(name="loads", bufs=2))
            bfp = ctx.enter_context(tc.tile_pool(name="bfp", bufs=2))
            tp = ctx.enter_context(tc.tile_pool(name="tp", bufs=2))
            expp = ctx.enter_context(tc.tile_pool(name="expp", bufs=2))
            otp = ctx.enter_context(tc.tile_pool(name="otp", bufs=2))
            outp = ctx.enter_context(tc.tile_pool(name="outp", bufs=2))
            rsp = ctx.enter_context(tc.tile_pool(name="rsp", bufs=2))
            ps_t = ctx.enter_context(tc.tile_pool(name="ps_t", bufs=2, space="PSUM"))
            ps_mm1 = ctx.enter_context(tc.tile_pool(name="ps_mm1", bufs=2, space="PSUM"))
            ps_ot = ctx.enter_context(tc.tile_pool(name="ps_ot", bufs=1, space="PSUM"))
            ps_out = ctx.enter_context(tc.tile_pool(name="ps_out", bufs=1, space="PSUM"))

            identb = consts.tile([P, P], bf16)
            make_identity(nc, identb)
            identf = consts.tile([P, P], f32)
            make_identity(nc, identf)
            embf = consts.tile([P, CHUNKS * DIM], f32)
            nc.sync.dma_start(out=embf, in_=de[:, :])

            for pr in range(PPC):
                # ---- load + k' = k + emb + bf16 convert (chunk-major) ----
                qf = loads.tile([P, CHUNKS * DIM], f32, tag="qf")
                kf = loads.tile([P, CHUNKS * DIM], f32, tag="kf")
                vf = loads.tile([P, CHUNKS * DIM], f32, tag="vf")
                nc.sync.dma_start(out=qf, in_=dq[pr, :, :])
                nc.sync.dma_start(out=kf, in_=dk[pr, :, :])
                nc.sync.dma_start(out=vf, in_=dv[pr, :, :])

                kpf = bfp.tile([P, CHUNKS * DIM], f32, tag="kpf")
                nc.vector.tensor_add(kpf, kf, embf)
                qb = bfp.tile([P, CHUNKS * DIM], bf16, tag="qb")
                nc.vector.tensor_copy(qb, qf)
                kb = bfp.tile([P, CHUNKS * DIM], bf16, tag="kb")
                nc.vector.tensor_copy(kb, kpf)
                # v' = [v | 1] per chunk: [128, 33*32] bf16
                vb = bfp.tile([P, CHUNKS, DIM + 1], bf16, tag="vb")
                nc.vector.memset(vb, 1.0)
                nc.vector.tensor_copy(
                    vb[:, :, 0:DIM],
                    vf.rearrange("p (c d) -> p c d", d=DIM),
                )

                # ---- PE transposes: qT/kT [64, 4096] bf16 (2 row-group replicas) ----
                qT = tp.tile([2 * DIM, SEQ], bf16, tag="qT")
                kT = tp.tile([2 * DIM, SEQ], bf16, tag="kT")
                for dst, src in ((qT, qb), (kT, kb)):
                    for half in range(4):
                        pst = ps_t.tile([2 * DIM, P * 8], bf16, tag="pst")
                        for cc in range(8):
                            c = 8 * half + cc
                            for r in range(2):
                                nc.tensor.transpose(
                                    pst[DIM * r : DIM * (r + 1), P * cc : P * (cc + 1)],
                                    src[:, DIM * c : DIM * (c + 1)],
                                    identb,
                                    tile_position=(0, DIM * r),
                                )
                        nc.vector.tensor_copy(
                            dst[:, P * 8 * half : P * 8 * (half + 1)], pst
                        )

                # ---- main loop over x blocks ----
                for w in range(XBLOCKS):
                    exb = expp.tile([P, CHUNKS * NB], bf16, tag="exb")
                    for g in range(CHUNKS // GROUP):
                        ps = ps_mm1.tile([P, GROUP * NB], mybir.dt.float32, tag="mm1")
                        for i in range(GROUP):
                            c = GROUP * g + i
                            nc.tensor.matmul(
                                ps[:, NB * i : NB * (i + 1)],
                                lhsT=kT[DIM * i : DIM * (i + 1), P * c : P * (c + 1)],
                                rhs=qT[DIM * i : DIM * (i + 1), NB * w : NB * (w + 1)],
                                start=True,
                                stop=True,
                                tile_position=(DIM * i, 0),
                            )
                        nc.scalar.activation(
                            exb[:, GROUP * NB * g : GROUP * NB * (g + 1)],
                            ps,
                            mybir.ActivationFunctionType.Exp,
                            scale=SCALE,
                        )

                    ot = ps_ot.tile([DIM + 1, NB], mybir.dt.float32, tag="ot")
                    exb3 = exb.rearrange("p (c n) -> p c n", n=NB)
                    for c in range(CHUNKS):
                        nc.tensor.matmul(
                            ot,
                            lhsT=vb[:, c, :],
                            rhs=exb3[:, c, :],
                            start=(c == 0),
                            stop=(c == CHUNKS - 1),
                        )
                    otsb = otp.tile([DIM + 1, NB], mybir.dt.float32, tag="otsb")
                    nc.vector.tensor_copy(otsb, ot)

                    outsb = outp.tile([P, NB // P, DIM], mybir.dt.float32, tag="outsb")
                    for s in range(NB // P):
                        po = ps_out.tile([P, DIM + 1], mybir.dt.float32, tag="po")
                        nc.tensor.transpose(
                            po,
                            otsb[:, P * s : P * (s + 1)],
                            identf[0 : DIM + 1, 0 : DIM + 1],
                        )
                        rs = rsp.tile([P, 1], mybir.dt.float32, tag="rs")
                        nc.vector.reciprocal(rs, po[:, DIM : DIM + 1])
                        nc.vector.tensor_scalar_mul(outsb[:, s, :], po[:, 0:DIM], rs)

                    nc.sync.dma_start(
                        out=do[pr, NB * w : NB * (w + 1), :].rearrange(
                            "(s p2) d -> p2 s d", p2=P
                        ),
                        in_=outsb,
                    )

    _split_excess_waits(nc, mybir)
    return nc


def _get_runner():
    if "nc" not in _RUNNER_CACHE:
        _RUNNER_CACHE["nc"] = _build_nc()
    return _RUNNER_CACHE["nc"]


def _host_prep(q, k, v, emb_h, emb_w):
    """Shard + rearrange inputs into per-core input maps."""
    q = np.ascontiguousarray(q, dtype=np.float32).reshape(BS * HEADS, SEQ, DIM)
    k = np.ascontiguousarray(k, dtype=np.float32).reshape(BS * HEADS, SEQ, DIM)
    v = np.ascontiguousarray(v, dtype=np.float32).reshape(BS * HEADS, SEQ, DIM)
    emb = (
        np.asarray(emb_h, dtype=np.float32)[:, None, :]
        + np.asarray(emb_w, dtype=np.float32)[None, :, :]
    ).reshape(SEQ, DIM)

    def cm(x):  # [..., 4096, 32] -> chunk-major [..., 128, 1024]
        lead = x.shape[:-2]
        return (
            x.reshape(*lead, CHUNKS, P, DIM)
            .swapaxes(-3, -2)
            .reshape(*lead, P, CHUNKS * DIM)
        )

    qcm, kcm, vcm, ecm = cm(q), cm(k), cm(v), cm(emb)
    in_maps = []
    for c in range(N_CORES):
        sl = slice(PPC * c, PPC * (c + 1))
        in_maps.append(
            {
                "q": np.ascontiguousarray(qcm[sl]),
                "k": np.ascontiguousarray(kcm[sl]),
                "v": np.ascontiguousarray(vcm[sl]),
                "emb": np.ascontiguousarray(ecm),
            }
        )
    return in_maps


def _host_gather(results):
    o_all = np.stack([results[c]["o"] for c in range(N_CORES)], axis=0).reshape(
        BS, HEADS, H, W, DIM
    )
    return np.ascontiguousarray(
        o_all.transpose(0, 2, 3, 1, 4).reshape(BS, H, W, HEADS * DIM)
    )


def run_on_hw(in_maps, trace=False):
    from concourse.bass_utils import run_bass_kernel_spmd

    nc = _get_runner()
    res = run_bass_kernel_spmd(nc, in_maps, core_ids=list(range(N_CORES)), trace=trace)
    return res


def kernel(q, k, v, emb_h, emb_w):
    in_maps = _host_prep(q, k, v, emb_h, emb_w)
    res = run_on_hw(in_maps, trace=False)
    return _host_gather(res.results)
